# revision 14
# baseline (speedup 1.0000x reference)
"""BrushStroke splat kernel for 8 trn2 NeuronCores.

out[b,c,y,x] = mean_n sum_{p,q} Fy[b,n,y,p] Fx[b,n,x,q] patches[b,n,c,p,q]
with Fx/Fy separable Gaussian filter banks (sigma=0.1) normalized over a
padded spatial axis.

Strategy (per core, 2 batches of 64 strokes):
 - The Gaussian is Toeplitz in (x - q): build one row E[n, t] =
   exp(-(t - center_n)^2 / (2 sigma^2)) of length 319 per stroke
   (ScalarE Square+Exp, strokes on partitions), then DMA-gather shifted
   windows into per-group filter tiles [(j,q'), x] (q' = reversed q).
 - MM1 per (group of 4 strokes, channel): one full-array f32r matmul
   with a block-diagonal lhsT [128,128] holding the 4 strokes' patch
   blocks scaled by the Fx normalizers -> t[(j,p'), x] in PSUM.
 - MM2 per (ytile, channel): 16 chained f32r matmuls accumulate
   sum_g FyN_g^T @ t_g into PSUM [y, x]; drain with x1/64.
Batch-parallel across cores; no collectives.
"""
import sys, types
import numpy as np

IMAGE = 256
PAD = 16
EPS = 1e-7
SIGMA2 = 2.0 * 0.1 ** 2
B, N, C, PH, PW = 16, 64, 3, 32, 32
NCORES = 8
BLOC = B // NCORES          # 2 batches per core
NG = N // 4                 # 16 groups of 4 strokes
ET = IMAGE + 2 * PAD + PW - 1   # 319: E row length


def _install_patches():
    if 'antenv.axon_hooks' not in sys.modules:
        mod = types.ModuleType('antenv.axon_hooks')
        mod._hook = None
        mod.set_axon_ntff_profile_hook = lambda h: setattr(mod, '_hook', h)
        mod.get_axon_ntff_profile_hook = lambda: mod._hook
        sys.modules['antenv.axon_hooks'] = mod
        try:
            from trn_agent_boot.trn_boot import _ntff_profile_via_ctypes
            hook = _ntff_profile_via_ctypes('/opt/axon/libaxon_pjrt.so')
            if hook is not None:
                mod.set_axon_ntff_profile_hook(hook)
        except Exception:
            pass

    import concourse.tile as tile
    import concourse.bass_utils as bass_utils
    from concourse.vector_clock import ScopedClock

    bass_utils.upload_artifacts = lambda tmpdir: 'local://' + tmpdir

    if getattr(tile.TileContext._drain_and_barrier, '_patched', False):
        return

    def _drain_and_barrier(self, tick_clock, wait_clock):
        nc = self.nc
        drain_inst = nc.sync.drain()
        wait_clock.add_sem_waits(
            drain_inst.ins, ScopedClock({None: tick_clock.global_clock}))
        si = drain_inst.ins.sync_info
        waits = list(si.on_wait or [])
        si.on_wait = []
        for w in waits:
            nop = nc.sync.nop()
            nop.ins.sync_info = type(si)(on_wait=[w], on_update=[])
        nc.all_engine_barrier()
        popped = nc._tile_sem_poison_stack.pop()
        assert popped is self._sem_poison
        nc.clear_and_free_semaphores(list(self.sems.allocated().values()))
        nc.all_engine_barrier()

    _drain_and_barrier._patched = True
    tile.TileContext._drain_and_barrier = _drain_and_barrier


def _split_multi_waits(nc):
    """This walrus accepts at most one sync wait per instruction; hoist
    extras onto same-engine NoOps inserted just before."""
    import bass_rust
    n_new = [0]

    def fresh_nop(engine, wait, si_type):
        n_new[0] += 1
        nop = bass_rust.InstNoOp(name=f'I-waitsplit-{n_new[0]}', ins=[], outs=[])
        nop.engine = engine
        nop.sync_info = si_type(on_wait=[wait], on_update=[])
        return nop

    for fn in nc.m.functions:
        for blk in fn.blocks:
            insts = blk.instructions
            i = 0
            while i < len(insts):
                inst = insts[i]
                si = inst.sync_info
                if si is not None and si.on_wait and len(si.on_wait) > 1:
                    waits = list(si.on_wait)
                    si.on_wait = [waits[-1]]
                    for k, w in enumerate(waits[:-1]):
                        insts.insert(i + k, fresh_nop(inst.engine, w, type(si)))
                    i += len(waits) - 1
                i += 1


_PROGRAM = None


def _build_program():
    global _PROGRAM
    if _PROGRAM is not None:
        return _PROGRAM
    _install_patches()
    import concourse.bass as bass
    import concourse.tile as tile
    from concourse import mybir
    from bass_rust import AP

    f32 = mybir.dt.float32
    f32r = mybir.dt.float32r
    AF = mybir.ActivationFunctionType
    AX = mybir.AxisListType

    nc = bass.Bass('TRN2', target_bir_lowering=False, debug=False,
                   num_devices=NCORES)
    # inputs (per core): brush coords by (batch,coord) rows; patches in
    # reversed-(p,q) block layout; 4x4 identity for the tiny transpose
    g_in = nc.declare_dram_parameter('g_in', [4, N], f32, isOutput=False)
    pt_in = nc.declare_dram_parameter('pt_in', [BLOC, NG, C, 128, 128], f32,
                                      isOutput=False)
    id4 = nc.declare_dram_parameter('id4', [4, 4], f32, isOutput=False)
    y_out = nc.declare_dram_parameter('y_out', [BLOC, C, IMAGE, IMAGE], f32,
                                      isOutput=True)

    with tile.TileContext(nc) as tc:
        with tc.tile_pool(name='glob', bufs=1) as gp, \
             tc.tile_pool(name='ps_init', bufs=1, space='PSUM') as psi:
            # ---- brush normalization -> centers bias vectors ----
            bc = gp.tile([4, N], f32)
            nc.sync.dma_start(bc[:], g_in[:])
            idt = gp.tile([4, 4], f32)
            nc.sync.dma_start(idt[:], id4[:])

            mn = gp.tile([4, 1], f32)
            mx = gp.tile([4, 1], f32)
            nc.vector.tensor_reduce(mn[:], bc[:], axis=AX.X,
                                    op=mybir.AluOpType.min)
            nc.vector.reduce_max(mx[:], bc[:], axis=AX.X)
            rng = gp.tile([4, 1], f32)
            nc.vector.tensor_sub(rng[:], mx[:], mn[:])
            nc.vector.tensor_scalar_add(rng[:], rng[:], EPS)
            inv = gp.tile([4, 1], f32)
            nc.vector.reciprocal(inv[:], rng[:])
            nc.vector.tensor_scalar_mul(inv[:], inv[:], float(IMAGE))
            gn = gp.tile([4, N], f32)
            nc.vector.tensor_scalar_sub(gn[:], bc[:], mn[:])
            nc.vector.tensor_scalar_mul(gn[:], gn[:], inv[:])

            # transpose [4,N] -> [N,4]; replicate to both 64-row halves
            tp_ps = psi.tile([N, 4], f32)
            nc.tensor.transpose(tp_ps[:], gn[:], idt[:])
            tp = gp.tile([128, 4], f32)
            nc.scalar.copy(tp[0:N, :], tp_ps[:])
            nc.vector.tensor_copy(tp[N:128, :], tp_ps[:])

            # bias = -(g + 31.5 or 31.6); rows 0:64 from cols 0/1 (batch0),
            # rows 64:128 from cols 2/3 (batch1)
            bias_x = gp.tile([128, 1], f32)
            bias_y = gp.tile([128, 1], f32)
            CX = PW / 2 - 0.5 + PAD      # 31.5
            CY = PW / 2 - 0.4 + PAD      # 31.6
            MUL, SUB = mybir.AluOpType.mult, mybir.AluOpType.subtract
            nc.vector.tensor_scalar(bias_x[0:N, :], tp[0:N, 0:1],
                                    -1.0, CX, MUL, SUB)
            nc.vector.tensor_scalar(bias_x[N:128, :], tp[N:128, 2:3],
                                    -1.0, CX, MUL, SUB)
            nc.vector.tensor_scalar(bias_y[0:N, :], tp[0:N, 1:2],
                                    -1.0, CY, MUL, SUB)
            nc.vector.tensor_scalar(bias_y[N:128, :], tp[N:128, 3:4],
                                    -1.0, CY, MUL, SUB)

            # ---- E rows: exp(-(t - c)^2 / SIGMA2), strokes on partitions --
            it = gp.tile([128, ET], f32)
            nc.gpsimd.iota(it[:], pattern=[[1, ET]], base=0,
                           channel_multiplier=0,
                           allow_small_or_imprecise_dtypes=True)
            sq = gp.tile([128, ET], f32)
            E_x = gp.tile([128, ET], f32r)
            E_y = gp.tile([128, ET], f32r)
            nc.scalar.activation(sq[:], it[:], AF.Square, bias=bias_x[:],
                                 scale=1.0)
            nc.scalar.activation(E_x[:], sq[:], AF.Exp, bias=0.0,
                                 scale=-1.0 / SIGMA2)
            sq2 = gp.tile([128, ET], f32)
            nc.scalar.activation(sq2[:], it[:], AF.Square, bias=bias_y[:],
                                 scale=1.0)
            nc.scalar.activation(E_y[:], sq2[:], AF.Exp, bias=0.0,
                                 scale=-1.0 / SIGMA2)

            # ---- per-batch main loop ----
            for b in range(BLOC):
                with tc.tile_pool(name=f'b{b}', bufs=1) as bp:
                    t_tiles = {}
                    fyn_tiles = {}
                    psa_cm = tc.tile_pool(name=f'ps_a{b}', bufs=2,
                                          space='PSUM')
                    psa = psa_cm.__enter__()
                    for g in range(NG):
                        pp = g % 2
                        # gathers: dest[(j,q'), x] = E[64b+4g+j, q' + x]
                        fx_g = bp.tile([128, ET - PW + 1], f32r,
                                       name=f'fxg{b}{pp}', tag=f'fxg{b}{pp}')
                        base = E_x[N * b + 4 * g: N * b + 4 * g + 1, 0:1]
                        src = AP(E_x.tensor, base.offset,
                                 [[ET, 4], [1, PW], [1, ET - PW + 1]])
                        nc.sync.dma_start(fx_g[:], src)
                        fy_g = bp.tile([128, ET - PW + 1], f32r,
                                       name=f'fyg{b}{pp}', tag=f'fyg{b}{pp}')
                        base = E_y[N * b + 4 * g: N * b + 4 * g + 1, 0:1]
                        src = AP(E_y.tensor, base.offset,
                                 [[ET, 4], [1, PW], [1, ET - PW + 1]])
                        nc.sync.dma_start(fy_g[:], src)

                        # window sums + reciprocal normalizers
                        wsx = bp.tile([128, 1], f32, name=f'wsx{b}{pp}', tag=f'wsx{b}{pp}')
                        scr = bp.tile([128, ET - PW + 1], f32r,
                                      name=f'scr{b}{pp}', tag=f'scr{b}{pp}')
                        nc.scalar.activation(scr[:], fx_g.bitcast(f32)[:],
                                             AF.Copy, scale=1.0,
                                             accum_out=wsx[:])
                        nc.vector.tensor_scalar_add(wsx[:], wsx[:], EPS)
                        invx = bp.tile([128, 1], f32, name=f'ivx{b}{pp}', tag=f'ivx{b}{pp}')
                        nc.vector.reciprocal(invx[:], wsx[:])

                        wsy = bp.tile([128, 1], f32, name=f'wsy{b}{pp}', tag=f'wsy{b}{pp}')
                        nc.vector.reduce_sum(wsy[:], fy_g.bitcast(f32)[:],
                                             axis=AX.X)
                        nc.vector.tensor_scalar_add(wsy[:], wsy[:], EPS)
                        invy = bp.tile([128, 1], f32, name=f'ivy{b}{pp}', tag=f'ivy{b}{pp}')
                        nc.vector.reciprocal(invy[:], wsy[:])

                        # normalized Fy [128, 256] f32r (y = 0..255)
                        fyn = bp.tile([128, IMAGE], f32r, name=f'fyn{b}_{g}', tag=f'fyn{b}_{g}')
                        nc.vector.tensor_scalar_mul(
                            fyn[:], fy_g.bitcast(f32)[:, PAD:PAD + IMAGE],
                            invy[:])
                        fyn_tiles[g] = fyn

                        # block-diagonal patch DMA (host pre-diagonalized)
                        # + normalize-scale (also the f32r rounding op)
                        praw = bp.tile([128, 128 * C], f32, name='praw',
                                       tag=f'praw{b}', bufs=2)
                        for c in range(C):
                            nc.sync.dma_start(
                                praw[:, 128 * c:128 * c + 128], pt_in[b, g, c])
                        ps_t = bp.tile([128, 128 * C], f32r, name='ptbds',
                                       tag=f'ptbds{b}', bufs=2)
                        nc.vector.tensor_scalar_mul(ps_t[:], praw[:], invx[:])

                        # MM1: one full-array f32r matmul per channel
                        for c in range(C):
                            p1 = psa.tile([128, IMAGE], f32, name=f'p1_{c}',
                                          tag=f'p1_{c}')
                            nc.tensor.matmul(
                                p1[:], ps_t[:, 128 * c:128 * c + 128],
                                fx_g[:, PAD:PAD + IMAGE],
                                start=True, stop=True)
                            tt = bp.tile([128, IMAGE], f32r,
                                         name=f't{b}_{g}_{c}', tag=f't{b}_{g}_{c}')
                            if c == 1:
                                nc.vector.tensor_copy(tt[:], p1[:])
                            else:
                                nc.scalar.copy(tt[:], p1[:])
                            t_tiles[(g, c)] = tt

                    psa_cm.__exit__(None, None, None)
                    # ---- MM2: accumulate over groups ----
                    with tc.tile_pool(name=f'ps_b{b}', bufs=2,
                                      space='PSUM') as psb:
                        for yt in range(2):
                            for c in range(C):
                                acc = psb.tile([128, IMAGE], f32,
                                               name=f'acc{c}', tag=f'acc{c}')
                                for g in range(NG):
                                    nc.tensor.matmul(
                                        acc[:],
                                        fyn_tiles[g][:, 128 * yt:128 * yt + 128],
                                        t_tiles[(g, c)][:],
                                        start=(g == 0), stop=(g == NG - 1))
                                ob = bp.tile([128, IMAGE], f32,
                                             name=f'ob{c}', tag=f'ob{c}')
                                nc.scalar.mul(ob[:], acc[:], 1.0 / N)
                                nc.sync.dma_start(
                                    y_out[b, c, 128 * yt:128 * yt + 128, :],
                                    ob[:])

    _split_multi_waits(nc)
    _PROGRAM = nc
    return nc


def _make_in_maps(brushes: np.ndarray, patches: np.ndarray):
    brushes = np.asarray(brushes, dtype=np.float32)
    patches = np.asarray(patches, dtype=np.float32)
    id4 = np.eye(4, dtype=np.float32)
    in_maps = []
    for k in range(NCORES):
        bsl = brushes[BLOC * k: BLOC * (k + 1)]        # [2, 64, 2]
        g_in = np.ascontiguousarray(
            bsl.transpose(0, 2, 1).reshape(4, N))       # rows b0x,b0y,b1x,b1y
        psl = patches[BLOC * k: BLOC * (k + 1)]         # [2, 64, 3, 32, 32]
        pr = psl.reshape(BLOC, NG, 4, C, PH, PW)[..., ::-1, ::-1]
        pr = pr.transpose(0, 1, 3, 2, 5, 4)             # [b, g, c, j, q', p']
        pt = np.zeros((BLOC, NG, C, 128, 128), np.float32)
        for j in range(4):
            pt[:, :, :, 32 * j:32 * j + 32, 32 * j:32 * j + 32] = pr[:, :, :, j]
        in_maps.append({'g_in': g_in, 'pt_in': pt, 'id4': id4})
    return in_maps


def kernel(brushes: np.ndarray, patches: np.ndarray) -> np.ndarray:
    from concourse.bass_utils import run_bass_kernel_spmd

    nc = _build_program()
    in_maps = _make_in_maps(brushes, patches)
    res = run_bass_kernel_spmd(nc, in_maps, list(range(NCORES)))
    out = np.concatenate([res.results[k]['y_out'] for k in range(NCORES)],
                         axis=0)
    return out


# revision 18
# speedup vs baseline: 1.5450x; 1.5450x over previous
"""BrushStroke splat kernel for 8 trn2 NeuronCores.

out[b,c,y,x] = mean_n sum_{p,q} Fy[b,n,y,p] Fx[b,n,x,q] patches[b,n,c,p,q]
with Fx/Fy separable Gaussian filter banks (sigma=0.1) normalized over a
padded spatial axis.

Strategy (per core, 2 batches of 64 strokes):
 - The Gaussian is Toeplitz in (x - q): build one row E[n, t] =
   exp(-(t - center_n)^2 / (2 sigma^2)) of length 319 per stroke
   (ScalarE Square+Exp, strokes on partitions), then DMA-gather shifted
   windows into per-group filter tiles [(j,q'), x] (q' = reversed q).
 - MM1 per (group of 4 strokes, channel): one full-array f32r matmul
   with a block-diagonal lhsT [128,128] holding the 4 strokes' patch
   blocks scaled by the Fx normalizers -> t[(j,p'), x] in PSUM.
 - MM2 per (ytile, channel): 16 chained f32r matmuls accumulate
   sum_g FyN_g^T @ t_g into PSUM [y, x]; drain with x1/64.
Batch-parallel across cores; no collectives.
"""
import sys, types
import numpy as np

IMAGE = 256
PAD = 16
EPS = 1e-7
SIGMA2 = 2.0 * 0.1 ** 2
B, N, C, PH, PW = 16, 64, 3, 32, 32
NCORES = 8
BLOC = B // NCORES          # 2 batches per core
NG = N // 4                 # 16 groups of 4 strokes
ET = IMAGE + 2 * PAD + PW - 1   # 319: E row length


def _install_patches():
    if 'antenv.axon_hooks' not in sys.modules:
        mod = types.ModuleType('antenv.axon_hooks')
        mod._hook = None
        mod.set_axon_ntff_profile_hook = lambda h: setattr(mod, '_hook', h)
        mod.get_axon_ntff_profile_hook = lambda: mod._hook
        sys.modules['antenv.axon_hooks'] = mod
        try:
            from trn_agent_boot.trn_boot import _ntff_profile_via_ctypes
            hook = _ntff_profile_via_ctypes('/opt/axon/libaxon_pjrt.so')
            if hook is not None:
                mod.set_axon_ntff_profile_hook(hook)
        except Exception:
            pass

    import concourse.tile as tile
    import concourse.bass_utils as bass_utils
    from concourse.vector_clock import ScopedClock

    bass_utils.upload_artifacts = lambda tmpdir: 'local://' + tmpdir

    if getattr(tile.TileContext._drain_and_barrier, '_patched', False):
        return

    def _drain_and_barrier(self, tick_clock, wait_clock):
        nc = self.nc
        drain_inst = nc.sync.drain()
        wait_clock.add_sem_waits(
            drain_inst.ins, ScopedClock({None: tick_clock.global_clock}))
        si = drain_inst.ins.sync_info
        waits = list(si.on_wait or [])
        si.on_wait = []
        for w in waits:
            nop = nc.sync.nop()
            nop.ins.sync_info = type(si)(on_wait=[w], on_update=[])
        nc.all_engine_barrier()
        popped = nc._tile_sem_poison_stack.pop()
        assert popped is self._sem_poison
        nc.clear_and_free_semaphores(list(self.sems.allocated().values()))
        nc.all_engine_barrier()

    _drain_and_barrier._patched = True
    tile.TileContext._drain_and_barrier = _drain_and_barrier


def _split_multi_waits(nc):
    """This walrus accepts at most one sync wait per instruction; hoist
    extras onto same-engine NoOps inserted just before."""
    import bass_rust
    n_new = [0]

    def fresh_nop(engine, wait, si_type):
        n_new[0] += 1
        nop = bass_rust.InstNoOp(name=f'I-waitsplit-{n_new[0]}', ins=[], outs=[])
        nop.engine = engine
        nop.sync_info = si_type(on_wait=[wait], on_update=[])
        return nop

    for fn in nc.m.functions:
        for blk in fn.blocks:
            insts = blk.instructions
            i = 0
            while i < len(insts):
                inst = insts[i]
                si = inst.sync_info
                if si is not None and si.on_wait and len(si.on_wait) > 1:
                    waits = list(si.on_wait)
                    si.on_wait = [waits[-1]]
                    for k, w in enumerate(waits[:-1]):
                        insts.insert(i + k, fresh_nop(inst.engine, w, type(si)))
                    i += len(waits) - 1
                i += 1


_PROGRAM = None


def _build_program():
    global _PROGRAM
    if _PROGRAM is not None:
        return _PROGRAM
    _install_patches()
    import concourse.bass as bass
    import concourse.tile as tile
    from concourse import mybir
    from bass_rust import AP

    f32 = mybir.dt.float32
    f32r = mybir.dt.float32r
    AF = mybir.ActivationFunctionType
    AX = mybir.AxisListType

    nc = bass.Bass('TRN2', target_bir_lowering=False, debug=False,
                   num_devices=NCORES)
    # inputs (per core): brush coords by (batch,coord) rows; patches in
    # reversed-(p,q) block layout; 4x4 identity for the tiny transpose
    g_in = nc.declare_dram_parameter('g_in', [4, N], f32, isOutput=False)
    pt_in = nc.declare_dram_parameter('pt_in', [BLOC, 128, NG * C * PH], f32,
                                      isOutput=False)
    id4 = nc.declare_dram_parameter('id4', [4, 4], f32, isOutput=False)
    y_out = nc.declare_dram_parameter('y_out', [BLOC, C, IMAGE, IMAGE], f32,
                                      isOutput=True)

    with tile.TileContext(nc) as tc:
        with tc.tile_pool(name='glob', bufs=1) as gp, \
             tc.tile_pool(name='ps_init', bufs=1, space='PSUM') as psi:
            # ---- brush normalization -> centers bias vectors ----
            bc = gp.tile([4, N], f32)
            nc.sync.dma_start(bc[:], g_in[:])
            idt = gp.tile([4, 4], f32)
            nc.sync.dma_start(idt[:], id4[:])

            mn = gp.tile([4, 1], f32)
            mx = gp.tile([4, 1], f32)
            nc.vector.tensor_reduce(mn[:], bc[:], axis=AX.X,
                                    op=mybir.AluOpType.min)
            nc.vector.reduce_max(mx[:], bc[:], axis=AX.X)
            rng = gp.tile([4, 1], f32)
            nc.vector.tensor_sub(rng[:], mx[:], mn[:])
            nc.vector.tensor_scalar_add(rng[:], rng[:], EPS)
            inv = gp.tile([4, 1], f32)
            nc.vector.reciprocal(inv[:], rng[:])
            nc.vector.tensor_scalar_mul(inv[:], inv[:], float(IMAGE))
            gn = gp.tile([4, N], f32)
            nc.vector.tensor_scalar_sub(gn[:], bc[:], mn[:])
            nc.vector.tensor_scalar_mul(gn[:], gn[:], inv[:])

            # transpose [4,N] -> [N,4]; replicate to both 64-row halves
            tp_ps = psi.tile([N, 4], f32)
            nc.tensor.transpose(tp_ps[:], gn[:], idt[:])
            tp = gp.tile([128, 4], f32)
            nc.scalar.copy(tp[0:N, :], tp_ps[:])
            nc.vector.tensor_copy(tp[N:128, :], tp_ps[:])

            # bias = -(g + 31.5 or 31.6); rows 0:64 from cols 0/1 (batch0),
            # rows 64:128 from cols 2/3 (batch1)
            bias_x = gp.tile([128, 1], f32)
            bias_y = gp.tile([128, 1], f32)
            CX = PW / 2 - 0.5 + PAD      # 31.5
            CY = PW / 2 - 0.4 + PAD      # 31.6
            MUL, SUB = mybir.AluOpType.mult, mybir.AluOpType.subtract
            nc.vector.tensor_scalar(bias_x[0:N, :], tp[0:N, 0:1],
                                    -1.0, CX, MUL, SUB)
            nc.vector.tensor_scalar(bias_x[N:128, :], tp[N:128, 2:3],
                                    -1.0, CX, MUL, SUB)
            nc.vector.tensor_scalar(bias_y[0:N, :], tp[0:N, 1:2],
                                    -1.0, CY, MUL, SUB)
            nc.vector.tensor_scalar(bias_y[N:128, :], tp[N:128, 3:4],
                                    -1.0, CY, MUL, SUB)

            # ---- E rows: exp(-(t - c)^2 / SIGMA2), strokes on partitions --
            it = gp.tile([128, ET], f32)
            nc.gpsimd.iota(it[:], pattern=[[1, ET]], base=0,
                           channel_multiplier=0,
                           allow_small_or_imprecise_dtypes=True)
            sq = gp.tile([128, ET], f32)
            E_x = gp.tile([128, ET], f32r)
            E_y = gp.tile([128, ET], f32r)
            nc.scalar.activation(sq[:], it[:], AF.Square, bias=bias_x[:],
                                 scale=1.0)
            nc.scalar.activation(E_x[:], sq[:], AF.Exp, bias=0.0,
                                 scale=-1.0 / SIGMA2)
            sq2 = gp.tile([128, ET], f32)
            nc.scalar.activation(sq2[:], it[:], AF.Square, bias=bias_y[:],
                                 scale=1.0)
            nc.scalar.activation(E_y[:], sq2[:], AF.Exp, bias=0.0,
                                 scale=-1.0 / SIGMA2)

            # ---- per-batch main loop ----
            for b in range(BLOC):
                with tc.tile_pool(name=f'b{b}', bufs=1) as bp:
                    # compact patches preload: [(j,q'), (g,c,p')]
                    ptc = bp.tile([128, NG * C * PH], f32, name=f'ptc{b}',
                                  tag=f'ptc{b}')
                    nc.sync.dma_start(ptc[:], pt_in[b])
                    # persistent ping-pong block-diagonal lhsT tiles; the
                    # off-diagonal zeros are written once and persist
                    ptbd = [bp.tile([128, 128 * C], f32r, name=f'ptbd{b}{i}',
                                    tag=f'ptbd{b}{i}') for i in range(2)]
                    for i in range(2):
                        nc.vector.memset(ptbd[i].bitcast(f32)[:], 0.0)
                    t_tiles = {}
                    fyn_tiles = {}
                    psa_cm = tc.tile_pool(name=f'ps_a{b}', bufs=2,
                                          space='PSUM')
                    psa = psa_cm.__enter__()
                    for g in range(NG):
                        pp = g % 2
                        # gathers: dest[(j,q'), x] = E[64b+4g+j, q' + x]
                        fx_g = bp.tile([128, ET - PW + 1], f32r,
                                       name=f'fxg{b}{pp}', tag=f'fxg{b}{pp}')
                        base = E_x[N * b + 4 * g: N * b + 4 * g + 1, 0:1]
                        src = AP(E_x.tensor, base.offset,
                                 [[ET, 4], [1, PW], [1, ET - PW + 1]])
                        nc.sync.dma_start(fx_g[:], src)
                        fy_g = bp.tile([128, ET - PW + 1], f32r,
                                       name=f'fyg{b}{pp}', tag=f'fyg{b}{pp}')
                        base = E_y[N * b + 4 * g: N * b + 4 * g + 1, 0:1]
                        src = AP(E_y.tensor, base.offset,
                                 [[ET, 4], [1, PW], [1, ET - PW + 1]])
                        nc.sync.dma_start(fy_g[:], src)

                        # window sums + reciprocal normalizers
                        wsx = bp.tile([128, 1], f32, name=f'wsx{b}{pp}', tag=f'wsx{b}{pp}')
                        scr = bp.tile([128, ET - PW + 1], f32r,
                                      name=f'scr{b}{pp}', tag=f'scr{b}{pp}')
                        nc.scalar.activation(scr[:], fx_g.bitcast(f32)[:],
                                             AF.Copy, scale=1.0,
                                             accum_out=wsx[:])
                        nc.vector.tensor_scalar_add(wsx[:], wsx[:], EPS)
                        invx = bp.tile([128, 1], f32, name=f'ivx{b}{pp}', tag=f'ivx{b}{pp}')
                        nc.vector.reciprocal(invx[:], wsx[:])

                        wsy = bp.tile([128, 1], f32, name=f'wsy{b}{pp}', tag=f'wsy{b}{pp}')
                        nc.vector.reduce_sum(wsy[:], fy_g.bitcast(f32)[:],
                                             axis=AX.X)
                        nc.vector.tensor_scalar_add(wsy[:], wsy[:], EPS)
                        invy = bp.tile([128, 1], f32, name=f'ivy{b}{pp}', tag=f'ivy{b}{pp}')
                        nc.vector.reciprocal(invy[:], wsy[:])

                        # normalized Fy [128, 256] f32r (y = 0..255)
                        fyn = bp.tile([128, IMAGE], f32r, name=f'fyn{b}_{g}', tag=f'fyn{b}_{g}')
                        nc.vector.tensor_scalar_mul(
                            fyn[:], fy_g.bitcast(f32)[:, PAD:PAD + IMAGE],
                            invy[:])
                        fyn_tiles[g] = fyn

                        # build block-diagonal lhsT on-chip: 4 partition-
                        # sliced strided copies, normalizer folded in (also
                        # the f32r rounding op)
                        ps_t = ptbd[g % 2]
                        for j in range(4):
                            dst0 = ps_t[32 * j:32 * j + 1, 32 * j:32 * j + 1]
                            dst = AP(ps_t.tensor, dst0.offset,
                                     [[128 * C, 32], [128, C], [1, PH]])
                            src0 = ptc[32 * j:32 * j + 1,
                                       96 * g:96 * g + 1]
                            srcap = AP(ptc.tensor, src0.offset,
                                       [[NG * C * PH, 32], [PH, C], [1, PH]])
                            nc.vector.tensor_scalar_mul(
                                dst, srcap, invx[32 * j:32 * j + 32, :])

                        # MM1: one full-array f32r matmul per channel
                        for c in range(C):
                            p1 = psa.tile([128, IMAGE], f32, name=f'p1_{c}',
                                          tag=f'p1_{c}')
                            nc.tensor.matmul(
                                p1[:], ps_t[:, 128 * c:128 * c + 128],
                                fx_g[:, PAD:PAD + IMAGE],
                                start=True, stop=True)
                            tt = bp.tile([128, IMAGE], f32r,
                                         name=f't{b}_{g}_{c}', tag=f't{b}_{g}_{c}')
                            if c == 1:
                                nc.vector.tensor_copy(tt[:], p1[:])
                            else:
                                nc.scalar.copy(tt[:], p1[:])
                            t_tiles[(g, c)] = tt

                    psa_cm.__exit__(None, None, None)
                    # ---- MM2: accumulate over groups ----
                    with tc.tile_pool(name=f'ps_b{b}', bufs=2,
                                      space='PSUM') as psb:
                        for yt in range(2):
                            for c in range(C):
                                acc = psb.tile([128, IMAGE], f32,
                                               name=f'acc{c}', tag=f'acc{c}')
                                for g in range(NG):
                                    nc.tensor.matmul(
                                        acc[:],
                                        fyn_tiles[g][:, 128 * yt:128 * yt + 128],
                                        t_tiles[(g, c)][:],
                                        start=(g == 0), stop=(g == NG - 1))
                                ob = bp.tile([128, IMAGE], f32,
                                             name=f'ob{c}', tag=f'ob{c}')
                                nc.scalar.mul(ob[:], acc[:], 1.0 / N)
                                nc.sync.dma_start(
                                    y_out[b, c, 128 * yt:128 * yt + 128, :],
                                    ob[:])

    _split_multi_waits(nc)
    _PROGRAM = nc
    return nc


def _make_in_maps(brushes: np.ndarray, patches: np.ndarray):
    brushes = np.asarray(brushes, dtype=np.float32)
    patches = np.asarray(patches, dtype=np.float32)
    id4 = np.eye(4, dtype=np.float32)
    in_maps = []
    for k in range(NCORES):
        bsl = brushes[BLOC * k: BLOC * (k + 1)]        # [2, 64, 2]
        g_in = np.ascontiguousarray(
            bsl.transpose(0, 2, 1).reshape(4, N))       # rows b0x,b0y,b1x,b1y
        psl = patches[BLOC * k: BLOC * (k + 1)]         # [2, 64, 3, 32, 32]
        pr = psl.reshape(BLOC, NG, 4, C, PH, PW)[..., ::-1, ::-1]
        # -> [b, j, q', g, c, p'] -> [b, 128, NG*C*PH]
        pt = np.ascontiguousarray(pr.transpose(0, 2, 5, 1, 3, 4)).reshape(
            BLOC, 128, NG * C * PH)
        in_maps.append({'g_in': g_in, 'pt_in': pt, 'id4': id4})
    return in_maps


def kernel(brushes: np.ndarray, patches: np.ndarray) -> np.ndarray:
    from concourse.bass_utils import run_bass_kernel_spmd

    nc = _build_program()
    in_maps = _make_in_maps(brushes, patches)
    res = run_bass_kernel_spmd(nc, in_maps, list(range(NCORES)))
    out = np.concatenate([res.results[k]['y_out'] for k in range(NCORES)],
                         axis=0)
    return out
